# revision 22
# baseline (speedup 1.0000x reference)
"""Trainium2 Bass kernel for nn_LogicLayer (differentiable logic-gate layer).

Reference computation:
    a = x[:, idx_a]; b = x[:, idx_b]                  # [B, OUT] gathers
    w = softmax(weights, -1)                          # [OUT, 16]
    out = sum_k w[:, k] * gate_k(a, b)

Every gate value is of the form c0 + c1*a + c2*b + c3*a*b, so
    out[i, j] = W0[j] + W1[j]*a + W2[j]*b + W3[j]*a*b
with W = softmax(weights) @ C, C the [16, 4] gate-coefficient table.

Kernel strategy (out_dim-parallel across 8 cores, 1024 neurons/core):
  host: W coefficients (softmax @ C, tiny), x transposed+cast to fp16
        xT16 [IN, B] passed as the gather table, per-core idx packing.
  device (per core, its 1024 j's, full batch on the free axis):
    1. dma_gather rows xT16[idx_a[j], :] and xT16[idx_b[j], :]
       (j on partitions, 4 KiB per gathered row -> efficient SWDGE DMA)
    2. s = W3*b + W1 (ACT), q = W2*b + W0 (DVE ts, 4x fp16 mode),
       m = a*s (DVE tt), o = m + q (DVE tt)
    3. store o to outT [1024, B] fp16 (4 KiB partition lines)
  host: assemble outT -> transpose -> float32 full output.

No PE/PSUM use at all and ~12 MiB HBM traffic per core vs ~41 MiB for
the batch-parallel transpose-on-device variant.
"""

import numpy as np

# ---------------------------------------------------------------- constants
B_TOT, IN_DIM, OUT_DIM = 2048, 8192, 8192
NCORES = 8
NJ_CORE = OUT_DIM // NCORES     # 1024 output neurons per core
CHUNK = 256                     # idxs per dma_gather call (2 slots of 128)

# value = c0 + c1*a + c2*b + c3*ab  for each of the 16 gates
GATE_C = np.array(
    [
        # c0  c1  c2  c3
        [0, 0, 0, 0],    # 0  False
        [0, 0, 0, 1],    # 1  a AND b
        [0, 1, 0, -1],   # 2  a AND NOT b
        [0, 1, 0, 0],    # 3  a
        [0, 0, 1, -1],   # 4  NOT a AND b
        [0, 0, 1, 0],    # 5  b
        [0, 1, 1, -2],   # 6  a XOR b
        [0, 1, 1, -1],   # 7  a OR b
        [1, -1, -1, 1],  # 8  NOT (a OR b)
        [1, -1, -1, 2],  # 9  NOT (a XOR b)
        [1, 0, -1, 0],   # 10 NOT b
        [1, 0, -1, 1],   # 11 a OR NOT b
        [1, -1, 0, 0],   # 12 NOT a
        [1, -1, 0, 1],   # 13 NOT a OR b
        [1, 0, 0, -1],   # 14 NOT (a AND b)
        [1, 0, 0, 0],    # 15 True
    ],
    dtype=np.float64,
)  # [16, 4]


# ---------------------------------------------------------------- device IR
def build_nc(NJ=NJ_CORE, IN=IN_DIM, B=B_TOT):
    """Build the per-core Bass module (SPMD; all cores run the same IR)."""
    import sys

    if "/opt/trn_rl_repo" not in sys.path:
        sys.path.insert(0, "/opt/trn_rl_repo")

    import concourse.tile as tile
    from concourse import bacc, mybir, library_config
    from contextlib import ExitStack

    f32 = mybir.dt.float32
    f16 = mybir.dt.float16
    u8 = mybir.dt.uint8
    i16 = mybir.dt.int16
    SLOTS = NJ // 128          # 8 j-slots per core
    # small first chunk -> compute ramps early; small last -> short tail
    CHUNKS = [128, 256, 256, 256, 128]
    assert sum(CHUNKS) == NJ

    nc = bacc.Bacc("TRN2", target_bir_lowering=False)
    xt = nc.declare_dram_parameter("xt16", [IN, B], f16, isOutput=False)
    wc = nc.declare_dram_parameter("wcoef", [128, 4 * SLOTS], f32, isOutput=False)
    ia = nc.declare_dram_parameter("idxa16", [128, NJ // 16], i16, isOutput=False)
    ib = nc.declare_dram_parameter("idxb16", [128, NJ // 16], i16, isOutput=False)
    outt = nc.declare_dram_parameter("outt", [NJ, B], u8, isOutput=True)

    Ident = mybir.ActivationFunctionType.Identity
    MULT = mybir.AluOpType.mult
    ADD = mybir.AluOpType.add

    with tile.TileContext(nc) as tc, ExitStack() as ctx:
        # kick the Q7 gather-lib swap off as early as possible: its ~9us
        # load latency gates the first dma_gather desc-gen
        nc.gpsimd.load_library(library_config.mlp)

        cpool = ctx.enter_context(tc.tile_pool(name="consts", bufs=1))
        iat = cpool.tile([128, NJ // 16], i16, name="iat")
        nc.sync.dma_start(iat[:], ia[:])
        ibt = cpool.tile([128, NJ // 16], i16, name="ibt")
        nc.sync.dma_start(ibt[:], ib[:])
        wct = cpool.tile([128, 4 * SLOTS], f32, name="wct")
        nc.sync.dma_start(wct[:], wc[:])

        # one MOVE per distinct chunk size instead of one per gather call
        # (each MOVE costs ~0.4us of GPSIMD sequencer time up front)
        nregs = {n: nc.gpsimd.to_reg(n) for n in sorted(set(CHUNKS))}

        gpool = ctx.enter_context(tc.tile_pool(name="gath", bufs=1))
        spool = ctx.enter_context(tc.tile_pool(name="sqm", bufs=3))
        opool = ctx.enter_context(tc.tile_pool(name="out", bufs=4))
        dpool = ctx.enter_context(tc.tile_pool(name="defer", bufs=1))

        def wap(k, c):  # [128, 1] f32 per-partition scalar for W_k, slot c
            return wct[:, k * SLOTS + c:k * SLOTS + c + 1]

        ACT_Q_SLOTS = {5, 6}   # late q's on ACT: it has slack there, DVE not
        POOL_O_SLOTS = set()   # (gpsimd tensor_tensor measured 4.7us/op: too slow; Pool is
        #                        done with desc-gen by then and otherwise idle
        last_c = NJ // 128 - 1
        deferred = []

        off = 0
        for ci, n in enumerate(CHUNKS):
            sl_n = n // 128
            icol0, icol1 = off // 16, (off + n) // 16
            # b feeds both s and q -> gather it first
            gb = gpool.tile([128, sl_n, B], f16, tag=f"gb{ci}")
            nc.gpsimd.dma_gather(
                gb[:], xt[:], ibt[:, icol0:icol1], n, nregs[n], B
            )
            ga = gpool.tile([128, sl_n, B], f16, tag=f"ga{ci}")
            nc.gpsimd.dma_gather(
                ga[:], xt[:], iat[:, icol0:icol1], n, nregs[n], B
            )
            for sl in range(sl_n):
                c = off // 128 + sl
                # final slot: split by batch halves to shorten the
                # un-overlapped dependence chain after the last gather
                hsplit = [slice(0, B // 2), slice(B // 2, B)] if c == last_c \
                    else [slice(0, B)]
                for hi, hs in enumerate(hsplit):
                    hb = hs.stop - hs.start
                    s = spool.tile([128, B], f16, tag="s")
                    nc.scalar.activation(
                        s[:, :hb], gb[:, sl, hs], Ident,
                        scale=wap(3, c), bias=wap(1, c),
                    )
                    qp = dpool if c in POOL_O_SLOTS else spool
                    q = qp.tile([128, B], f16, tag=f"qd{c}" if c in POOL_O_SLOTS else "q")
                    if c in ACT_Q_SLOTS:
                        nc.scalar.activation(
                            q[:, :hb], gb[:, sl, hs], Ident,
                            scale=wap(2, c), bias=wap(0, c),
                        )
                    else:
                        nc.vector.tensor_scalar(
                            q[:, :hb], gb[:, sl, hs], wap(2, c), wap(0, c),
                            op0=MULT, op1=ADD,
                        )
                    qknown = q  # keep name for deferred capture
                    mp = dpool if c in POOL_O_SLOTS else spool
                    m = mp.tile([128, B], f16, tag=f"md{c}" if c in POOL_O_SLOTS else "m")
                    nc.vector.tensor_tensor(
                        m[:, :hb], ga[:, sl, hs], s[:, :hb], op=MULT
                    )
                    if c in POOL_O_SLOTS:
                        deferred.append((c, hs, hb, m, qknown))
                        continue
                    o = opool.tile([128, B], u8, tag=f"o{hi}")
                    nc.vector.tensor_tensor(
                        o[:, :hb], m[:, :hb], q[:, :hb], op=ADD
                    )
                    nc.sync.dma_start(
                        outt[c * 128:(c + 1) * 128, hs], o[:, :hb]
                    )
            off += n

        # deferred o-adds on GPSIMD, after every dma_gather in program order
        for c, hs, hb, m, q in deferred:
            o = dpool.tile([128, B], u8, tag=f"op{c}")
            nc.gpsimd.tensor_tensor(o[:, :hb], m[:, :hb], q[:, :hb], op=ADD)
            nc.sync.dma_start(outt[c * 128:(c + 1) * 128, hs], o[:, :hb])
    nc.compile()
    return nc


# ---------------------------------------------------------------- host side
def _wrap_idx(idx):
    """Pack an index vector into dma_gather's wrapped int16 layout:
    idx16[p, s] = idx[s*16 + p%16], replicated over the 8 groups of 16
    partitions."""
    n = len(idx)
    a = np.asarray(idx).astype(np.int16).reshape(n // 16, 16)  # [s, p]
    return np.ascontiguousarray(np.tile(a.T, (8, 1)))          # [128, n//16]


def _prep_inputs(x, weights, idx_a, idx_b):
    x = np.asarray(x, dtype=np.float32)
    w = np.asarray(weights, dtype=np.float64)
    e = np.exp(w - w.max(axis=-1, keepdims=True))
    sm = e / e.sum(axis=-1, keepdims=True)
    W4 = (sm @ GATE_C)                                         # [OUT, 4]
    # fold the uint8 output quantization out_u8 = 254*out + 0.5 into W:
    # s = 254*W3*b + 254*W1, q = 254*W2*b + (254*W0 + 0.5), o = a*s + q
    W4 = W4 * 254.0
    W4[:, 0] += 0.5
    W4 = W4.astype(np.float32)

    xt16 = x.T.astype(np.float16, order="C")                   # [IN, B]
    idx_a = np.asarray(idx_a)
    idx_b = np.asarray(idx_b)

    SLOTS = NJ_CORE // 128
    in_maps = []
    for c in range(NCORES):
        j0 = c * NJ_CORE
        # wcoef[q, k*SLOTS + c] = W4[j0 + c*128 + q, k]
        wcoef = np.ascontiguousarray(
            W4[j0:j0 + NJ_CORE]
            .reshape(SLOTS, 128, 4)
            .transpose(1, 2, 0)
            .reshape(128, 4 * SLOTS)
        )
        in_maps.append(
            {
                "xt16": xt16,
                "wcoef": wcoef,
                "idxa16": _wrap_idx(idx_a[j0:j0 + NJ_CORE]),
                "idxb16": _wrap_idx(idx_b[j0:j0 + NJ_CORE]),
            }
        )
    return in_maps


_NC_CACHE = {}


def _get_nc():
    if "nc" not in _NC_CACHE:
        _NC_CACHE["nc"] = build_nc()
    return _NC_CACHE["nc"]


def _post(res, inputs=None):
    outt = np.concatenate([r["outt"] for r in res.results], axis=0)  # [OUT, B]
    return ((outt.T.astype(np.float32, order="C")) - 0.5) * (1.0 / 254.0)


def kernel(x, weights, idx_a, idx_b):
    import sys

    if "/opt/trn_rl_repo" not in sys.path:
        sys.path.insert(0, "/opt/trn_rl_repo")
    from concourse.bass_utils import run_bass_kernel_spmd

    nc = _get_nc()
    in_maps = _prep_inputs(x, weights, idx_a, idx_b)
    res = run_bass_kernel_spmd(nc, in_maps, list(range(NCORES)))
    return _post(res)


if __name__ == "__main__":
    nc = build_nc()
    print("built OK")


# revision 24
# speedup vs baseline: 1.0311x; 1.0311x over previous
"""Trainium2 Bass kernel for nn_LogicLayer (differentiable logic-gate layer).

Reference computation:
    a = x[:, idx_a]; b = x[:, idx_b]                  # [B, OUT] gathers
    w = softmax(weights, -1)                          # [OUT, 16]
    out = sum_k w[:, k] * gate_k(a, b)

Every gate value is of the form c0 + c1*a + c2*b + c3*a*b, so
    out[i, j] = W0[j] + W1[j]*a + W2[j]*b + W3[j]*a*b
with W = softmax(weights) @ C, C the [16, 4] gate-coefficient table.

Kernel strategy (out_dim-parallel across 8 cores, 1024 neurons/core):
  host: W coefficients (softmax @ C, tiny), x transposed+cast to fp16
        xT16 [IN, B] passed as the gather table, per-core idx packing.
  device (per core, its 1024 j's, full batch on the free axis):
    1. dma_gather rows xT16[idx_a[j], :] and xT16[idx_b[j], :]
       (j on partitions, 4 KiB per gathered row -> efficient SWDGE DMA)
    2. s = W3*b + W1 (ACT), q = W2*b + W0 (DVE ts, 4x fp16 mode),
       m = a*s (DVE tt), o = m + q (DVE tt)
    3. store o to outT [1024, B] fp16 (4 KiB partition lines)
  host: assemble outT -> transpose -> float32 full output.

No PE/PSUM use at all and ~12 MiB HBM traffic per core vs ~41 MiB for
the batch-parallel transpose-on-device variant.
"""

import numpy as np

# ---------------------------------------------------------------- constants
B_TOT, IN_DIM, OUT_DIM = 2048, 8192, 8192
NCORES = 8
NJ_CORE = OUT_DIM // NCORES     # 1024 output neurons per core
CHUNK = 256                     # idxs per dma_gather call (2 slots of 128)

# value = c0 + c1*a + c2*b + c3*ab  for each of the 16 gates
GATE_C = np.array(
    [
        # c0  c1  c2  c3
        [0, 0, 0, 0],    # 0  False
        [0, 0, 0, 1],    # 1  a AND b
        [0, 1, 0, -1],   # 2  a AND NOT b
        [0, 1, 0, 0],    # 3  a
        [0, 0, 1, -1],   # 4  NOT a AND b
        [0, 0, 1, 0],    # 5  b
        [0, 1, 1, -2],   # 6  a XOR b
        [0, 1, 1, -1],   # 7  a OR b
        [1, -1, -1, 1],  # 8  NOT (a OR b)
        [1, -1, -1, 2],  # 9  NOT (a XOR b)
        [1, 0, -1, 0],   # 10 NOT b
        [1, 0, -1, 1],   # 11 a OR NOT b
        [1, -1, 0, 0],   # 12 NOT a
        [1, -1, 0, 1],   # 13 NOT a OR b
        [1, 0, 0, -1],   # 14 NOT (a AND b)
        [1, 0, 0, 0],    # 15 True
    ],
    dtype=np.float64,
)  # [16, 4]


# ---------------------------------------------------------------- device IR
def build_nc(NJ=NJ_CORE, IN=IN_DIM, B=B_TOT):
    """Build the per-core Bass module (SPMD; all cores run the same IR)."""
    import sys

    if "/opt/trn_rl_repo" not in sys.path:
        sys.path.insert(0, "/opt/trn_rl_repo")

    import concourse.tile as tile
    from concourse import bacc, mybir, library_config
    from contextlib import ExitStack

    f32 = mybir.dt.float32
    f16 = mybir.dt.float16
    i16 = mybir.dt.int16
    SLOTS = NJ // 128          # 8 j-slots per core

    nc = bacc.Bacc("TRN2", target_bir_lowering=False)
    xt = nc.declare_dram_parameter("xt16", [IN, B], f16, isOutput=False)
    wc = nc.declare_dram_parameter("wcoef", [128, 4 * SLOTS], f32, isOutput=False)
    ix = nc.declare_dram_parameter("idx16", [128, 2 * NJ // 16], i16, isOutput=False)
    outt = nc.declare_dram_parameter("outt", [NJ, B], f16, isOutput=True)

    Ident = mybir.ActivationFunctionType.Identity
    MULT = mybir.AluOpType.mult
    ADD = mybir.AluOpType.add

    with tile.TileContext(nc) as tc, ExitStack() as ctx:
        # kick the Q7 gather-lib swap off as early as possible: its ~9us
        # load latency gates the first dma_gather desc-gen
        nc.gpsimd.load_library(library_config.mlp)

        cpool = ctx.enter_context(tc.tile_pool(name="consts", bufs=1))
        ixt = cpool.tile([128, 2 * NJ // 16], i16, name="ixt")
        nc.sync.dma_start(ixt[:], ix[:])
        wct = cpool.tile([128, 4 * SLOTS], f32, name="wct")
        nc.sync.dma_start(wct[:], wc[:])

        # one MOVE per distinct gather size instead of one per call
        nregs = {n: nc.gpsimd.to_reg(n) for n in (128, 512)}

        gpool = ctx.enter_context(tc.tile_pool(name="gath", bufs=1))
        spool = ctx.enter_context(tc.tile_pool(name="sqm", bufs=3))
        opool = ctx.enter_context(tc.tile_pool(name="out", bufs=4))

        def wap(k, c):  # [128, 1] f32 per-partition scalar for W_k, slot c
            return wct[:, k * SLOTS + c:k * SLOTS + c + 1]

        ACT_Q_SLOTS = {2, 3}   # early-mid q's on ACT slack; never before tail s
        last_c = NJ // 128 - 1

        # gather plan: 7 calls (under the 8-deep SWDGE sem pool).
        # first/last chunks fetch b and a separately (early s/q start, early
        # tail); middle chunks fetch a||b combined.
        # entries: (kind, slot_lo, n_slots) with kind in {"b","a","ab"}
        plan = [("b", 0, 1), ("a", 0, 1),
                ("ab", 1, 2), ("ab", 3, 2), ("ab", 5, 2),
                ("b", 7, 1), ("a", 7, 1)]
        gtiles = {}   # (kind in {"a","b"}, slot) -> (tile, sub-slot)
        icol = 0
        for kind, lo, ns in plan:
            nidx = (2 if kind == "ab" else 1) * ns * 128
            nsub = nidx // 128
            gt = gpool.tile([128, nsub, B], f16, tag=f"g{kind}{lo}")
            nc.gpsimd.dma_gather(
                gt[:], xt[:], ixt[:, icol:icol + nidx // 16],
                nidx, nregs[nidx], B
            )
            icol += nidx // 16
            for k in range(ns):
                if kind in ("a", "ab"):
                    gtiles[("a", lo + k)] = (gt, k)
                if kind in ("b", "ab"):
                    gtiles[("b", lo + k)] = (gt, (ns + k) if kind == "ab" else k)

            # emit compute for every slot whose a AND b are now requested
            done = [c for c in range(lo, lo + ns)
                    if ("a", c) in gtiles and ("b", c) in gtiles]
            for c in done:
                gat, ka = gtiles[("a", c)]
                gbt, kb = gtiles[("b", c)]
                ga, gb = gat[:, ka, :], gbt[:, kb, :]
                hsplit = [slice(0, B // 2), slice(B // 2, B)] if c == last_c \
                    else [slice(0, B)]
                for hi, hs in enumerate(hsplit):
                    hb = hs.stop - hs.start
                    s = spool.tile([128, B], f16, tag="s")
                    nc.scalar.activation(
                        s[:, :hb], gb[:, hs], Ident,
                        scale=wap(3, c), bias=wap(1, c),
                    )
                    q = spool.tile([128, B], f16, tag="q")
                    if c in ACT_Q_SLOTS:
                        nc.scalar.activation(
                            q[:, :hb], gb[:, hs], Ident,
                            scale=wap(2, c), bias=wap(0, c),
                        )
                    else:
                        nc.vector.tensor_scalar(
                            q[:, :hb], gb[:, hs], wap(2, c), wap(0, c),
                            op0=MULT, op1=ADD,
                        )
                    m = spool.tile([128, B], f16, tag="m")
                    nc.vector.tensor_tensor(
                        m[:, :hb], ga[:, hs], s[:, :hb], op=MULT
                    )
                    o = opool.tile([128, B], f16, tag=f"o{hi}")
                    nc.vector.tensor_tensor(
                        o[:, :hb], m[:, :hb], q[:, :hb], op=ADD
                    )
                    nc.sync.dma_start(
                        outt[c * 128:(c + 1) * 128, hs], o[:, :hb]
                    )
    nc.compile()
    return nc


# ---------------------------------------------------------------- host side
def _wrap_block(idx):
    """Pack one call's index list into dma_gather's wrapped int16 layout:
    idx16[p, s] = idx[s*16 + p%16], replicated over the 8 groups of 16
    partitions."""
    n = len(idx)
    a = np.asarray(idx).astype(np.int16).reshape(n // 16, 16)  # [s, p]
    return np.tile(a.T, (8, 1))                                # [128, n//16]


def _pack_idx(ia, ib):
    """Mirror the device's 7-call gather plan (see build_nc)."""
    blocks = [
        _wrap_block(ib[0:128]), _wrap_block(ia[0:128]),
        _wrap_block(np.concatenate([ia[128:384], ib[128:384]])),
        _wrap_block(np.concatenate([ia[384:640], ib[384:640]])),
        _wrap_block(np.concatenate([ia[640:896], ib[640:896]])),
        _wrap_block(ib[896:1024]), _wrap_block(ia[896:1024]),
    ]
    return np.ascontiguousarray(np.concatenate(blocks, axis=1))


def _prep_inputs(x, weights, idx_a, idx_b):
    x = np.asarray(x, dtype=np.float32)
    w = np.asarray(weights, dtype=np.float64)
    e = np.exp(w - w.max(axis=-1, keepdims=True))
    sm = e / e.sum(axis=-1, keepdims=True)
    W4 = (sm @ GATE_C).astype(np.float32)                      # [OUT, 4]

    xt16 = x.T.astype(np.float16, order="C")                   # [IN, B]
    idx_a = np.asarray(idx_a)
    idx_b = np.asarray(idx_b)

    SLOTS = NJ_CORE // 128
    in_maps = []
    for c in range(NCORES):
        j0 = c * NJ_CORE
        # wcoef[q, k*SLOTS + c] = W4[j0 + c*128 + q, k]
        wcoef = np.ascontiguousarray(
            W4[j0:j0 + NJ_CORE]
            .reshape(SLOTS, 128, 4)
            .transpose(1, 2, 0)
            .reshape(128, 4 * SLOTS)
        )
        in_maps.append(
            {
                "xt16": xt16,
                "wcoef": wcoef,
                "idx16": _pack_idx(idx_a[j0:j0 + NJ_CORE],
                                   idx_b[j0:j0 + NJ_CORE]),
            }
        )
    return in_maps


_NC_CACHE = {}


def _get_nc():
    if "nc" not in _NC_CACHE:
        _NC_CACHE["nc"] = build_nc()
    return _NC_CACHE["nc"]


def _post(res, inputs=None):
    outt = np.concatenate([r["outt"] for r in res.results], axis=0)  # [OUT, B]
    return outt.T.astype(np.float32, order="C")


def kernel(x, weights, idx_a, idx_b):
    import sys

    if "/opt/trn_rl_repo" not in sys.path:
        sys.path.insert(0, "/opt/trn_rl_repo")
    from concourse.bass_utils import run_bass_kernel_spmd

    nc = _get_nc()
    in_maps = _prep_inputs(x, weights, idx_a, idx_b)
    res = run_bass_kernel_spmd(nc, in_maps, list(range(NCORES)))
    return _post(res)


if __name__ == "__main__":
    nc = build_nc()
    print("built OK")
